# revision 30
# baseline (speedup 1.0000x reference)
"""Trainium2 Bass kernel for nn_DisAttLayer (disentangled-attention score bias).

Math (per batch b, head h, query m, key n):
    z1[k]  = A[k, m] + B[k, n] + C[k, m-n+256]          (layer-1 decomposed)
    e1     = relu(z1)
    z2[l]  = sum_k e1[k] * w2[k, l, h]
    e2     = relu(z2)
    out    = attn[b,h,m,n] + sum_l e2[l] * w3[l, h]

where A/B/C are tiny per-row projections of the gathered embeddings:
    A[.,m] = bi[m] @ w1[32:48] + ci[m] @ w1[64:80]      (i-side features)
    B[.,n] = bj[n] @ w1[48:64] + cj[n] @ w1[80:96]      (j-side features)
    C[.,d] = e_pos[d] @ w1[0:32]                        (relative position)

Sharding: 8 cores = 4 batches x 2 m-halves. One SPMD NEFF; the per-core
m-offset is absorbed into a host-relayouted (shifted+reversed) e_pos table
so the program is identical on every core.

Raw-bass implementation (manual semaphores): this container's neuronx-cc
allows at most ONE sync wait per instruction struct, which Tile-generated
sync does not respect; with explicit per-engine programs every dependency
is expressed as standalone wait_ge instructions ahead of the consuming op.

Engine plan per core (128 m x 256 n x 8 heads, two 4-head groups):
  SP   : weight loads (HWDGE), per-block output stores
  ACT  : idx/epos/attn loads (HWDGE queue), f32->bf16 casts, psum drains,
         relu(z1+A) for some m, e2 relu-drains, score drains
  Pool : indirect embedding gathers (SWDGE), C+B adds (odd m), some
         relu(z1+A), attn add
  DVE  : C+B adds (even m), most relu(z1+A)
  PE   : feature transposes (matmul vs identity), A/B/C projections,
         layer-2 matmuls (both groups packed via tile_position), layer-3
         col-tiled accumulation into [m*8+h, n] score psum
"""

import os
import sys

import numpy as np

sys.path.insert(0, "/opt/trn_rl_repo")

B, S, H = 4, 256, 8
N_MB, N_C = 50000, 1000
P, BS, CS, L0, L1, L2 = 32, 16, 16, 96, 32, 16
M = 128          # m-rows per core
N = 256          # n-cols per core
MB = 16          # m-rows per main-loop block
NBLK = M // MB   # 8 blocks
NCH = MB // 2    # 512-cell chunks per block
NCORES = 8

# pass-2 (relu(z1+A)) engine assignment per (group, m-within-block)
A_I = {0: (5, 7, 9), 1: (5, 7, 9, 11)}
G_I = (1, 3)


def _p2eng(g, i):
    if i in G_I:
        return "g"
    if i in A_I[g]:
        return "a"
    return "v"


V_I = {g: [i for i in range(MB) if _p2eng(g, i) == "v"] for g in (0, 1)}

LAST_RESULT = None
_BUILT = None


def _build_nc():
    from contextlib import ExitStack

    import concourse.bass as bass
    import concourse.mybir as mybir

    F32 = mybir.dt.float32
    BF16 = mybir.dt.bfloat16
    I32 = mybir.dt.int32
    AL = mybir.AluOpType
    AF = mybir.ActivationFunctionType

    nc = bass.Bass(
        "TRN2",
        target_bir_lowering=False,
        debug=False,
        enable_asserts=False,
        num_devices=NCORES,
    )

    d_attn = nc.dram_tensor("t_attn", [H, M, N], F32, kind="ExternalInput").ap()
    d_ebi = nc.dram_tensor("t_ebi", [N_MB + 1, H * BS], F32, kind="ExternalInput").ap()
    d_ebj = nc.dram_tensor("t_ebj", [N_MB + 1, H * BS], F32, kind="ExternalInput").ap()
    d_eci = nc.dram_tensor("t_eci", [N_C + 2, H * CS], F32, kind="ExternalInput").ap()
    d_ecj = nc.dram_tensor("t_ecj", [N_C + 2, H * CS], F32, kind="ExternalInput").ap()
    d_bi_idx = nc.dram_tensor("t_bi_idx", [M, 1], I32, kind="ExternalInput").ap()
    d_ci_idx = nc.dram_tensor("t_ci_idx", [M, 1], I32, kind="ExternalInput").ap()
    d_bj_idx = nc.dram_tensor("t_bj_idx", [N, 1], I32, kind="ExternalInput").ap()
    d_cj_idx = nc.dram_tensor("t_cj_idx", [N, 1], I32, kind="ExternalInput").ap()
    d_w1bi = nc.dram_tensor("t_w1bi", [128, 128], F32, kind="ExternalInput").ap()
    d_w1ci = nc.dram_tensor("t_w1ci", [128, 128], F32, kind="ExternalInput").ap()
    d_w1bj = nc.dram_tensor("t_w1bj", [128, 128], F32, kind="ExternalInput").ap()
    d_w1cj = nc.dram_tensor("t_w1cj", [128, 128], F32, kind="ExternalInput").ap()
    d_ident = nc.dram_tensor("t_ident", [128, 128], F32, kind="ExternalInput").ap()
    d_w1p = nc.dram_tensor("t_w1p", [2, 128, 128], F32, kind="ExternalInput").ap()
    d_w2 = nc.dram_tensor("t_w2", [2, 128, 64], F32, kind="ExternalInput").ap()
    d_w3m = nc.dram_tensor("t_w3m", [4, 128, 32], F32, kind="ExternalInput").ap()
    d_epos = nc.dram_tensor("t_epos", [2, 128, 512], F32, kind="ExternalInput").ap()
    d_out = nc.dram_tensor("t_out", [H, M, N], F32, kind="ExternalOutput").ap()

    def sub_ap(t_ap, elem_off, dims):
        return bass.AP(tensor=t_ap.tensor, offset=t_ap.offset + elem_off, ap=list(dims))

    def mh_dram_ap(dram_ap, mb0):
        # iterate (m-within-block, h, n) to match score partitions p = m*8+h
        return sub_ap(dram_ap, mb0 * N, [[N, MB], [M * N, H], [1, N]])

    # ---- tick formulas (sem value after the k-th op on each engine) ----
    SP_LOADS = ["w1bi", "w1ci", "w1bj", "w1cj", "w1p0", "w1p1",
                "w2_0", "w2_1", "w3m0", "w3m1", "w3m2", "w3m3", "ident"]
    sp_tick = {nm: 16 * (k + 1) for k, nm in enumerate(SP_LOADS)}
    AQ_LOADS = ["bi_idx", "ci_idx", "bj_idx0", "bj_idx1", "cj_idx0", "cj_idx1",
                "epos0", "epos1", "attn"]
    aq_tick = {nm: 16 * (k + 1) for k, nm in enumerate(AQ_LOADS)}
    GATHERS = ["bi", "ci", "bj0", "bj1", "cj0", "cj1"]
    gq_tick = {nm: 16 * (k + 1) for k, nm in enumerate(GATHERS)}
    # ACT compute ticks: 14 casts, 6 transpose drains, 8 A/B/C drains, then
    # per block: len(A_I[0])+len(A_I[1]) pass2-a + NCH e2 drains + 1 score
    CASTS = ["w1bi", "w1ci", "w1bj", "w1cj", "w1p0", "w1p1", "w2_0", "w2_1",
             "w3m0", "w3m1", "w3m2", "w3m3", "epos0", "epos1"]
    cast_tick = {nm: k + 1 for k, nm in enumerate(CASTS)}
    xd_tick = {k: 15 + k for k in range(6)}          # bi, ci, bj0, bj1, cj0, cj1
    dr_tick = {}
    t = 20
    for g in (0, 1):
        for w in ("A", "B", "C0", "C1"):
            t += 1
            dr_tick[(g, w)] = t
    ACT_SETUP = t  # 28
    NA = len(A_I[0]) + len(A_I[1])                   # 7
    ACT_PER_BLK = NA + NCH + 1                       # 16

    def act_tsa(blk, g, i):
        pos = A_I[g].index(i) + 1 + (0 if g == 0 else len(A_I[0]))
        return ACT_SETUP + blk * ACT_PER_BLK + pos

    def act_e2d(cg):
        blk, c = divmod(cg, NCH)
        return ACT_SETUP + blk * ACT_PER_BLK + NA + 1 + c

    def act_scored(blk):
        return ACT_SETUP + blk * ACT_PER_BLK + ACT_PER_BLK

    DVE_PER_BLK = 2 + len(V_I[0]) + len(V_I[1])

    def dve_tt1e(blk, g):
        return blk * DVE_PER_BLK + (1 if g == 0 else 2 + len(V_I[0]))

    def dve_tsv(blk, g, i):
        return dve_tt1e(blk, g) + V_I[g].index(i) + 1

    # Pool compute ticks: blk0 has [tt1o(0), tsg(0,1), tsg(0,3), tt1o(1),
    # tsg(1,1), tsg(1,3)]; blk>=1 prepends res(blk-1); res(7) last
    def pool_base(blk):
        return 0 if blk == 0 else 6 + 7 * (blk - 1)

    def pool_tt1o(blk, g):
        return pool_base(blk) + (0 if blk == 0 else 1) + (1 if g == 0 else 4)

    def pool_tsg(blk, g, i):
        return pool_tt1o(blk, g) + (1 if i == G_I[0] else 2)

    def pool_res(blk):
        if blk < NBLK - 1:
            return pool_base(blk + 1) + 1
        return pool_base(NBLK - 1) + 7 + 1

    # PE ticks: 6 transposes, then per g: 4 AB + 1 C, then per chunk cg:
    # [MM2g0, MM2g1, MM3a, MM3b]
    PE_SETUP = 6 + 10

    def pe_mm2g1(cg):
        return PE_SETUP + cg * 4 + 2

    def pe_mm3b(cg):
        return PE_SETUP + cg * 4 + 4

    def ts_tick(blk, g, i):
        e = _p2eng(g, i)
        if e == "v":
            return ("dve", dve_tsv(blk, g, i))
        if e == "a":
            return ("act", act_tsa(blk, g, i))
        return ("pool", pool_tsg(blk, g, i))

    with ExitStack() as ctx:
        ent = ctx.enter_context

        def sb(name, shape, dtype):
            return ent(nc.sbuf_tensor(name, shape, dtype)).ap()

        w_f = {nm: sb(f"f_{nm}", shp, F32) for nm, shp in
               [("w1bi", [128, 128]), ("w1ci", [128, 128]), ("w1bj", [128, 128]),
                ("w1cj", [128, 128]), ("w1p0", [128, 128]), ("w1p1", [128, 128]),
                ("w2_0", [128, 64]), ("w2_1", [128, 64]), ("w3m0", [128, 32]),
                ("w3m1", [128, 32]), ("w3m2", [128, 32]), ("w3m3", [128, 32]),
                ("epos0", [128, 512]), ("epos1", [128, 512])]}
        ident = sb("ident", [128, 128], F32)
        idx_sb = {nm: sb(f"i_{nm}", [128, 1], I32) for nm in
                  ["bi_idx", "ci_idx", "bj_idx0", "bj_idx1", "cj_idx0", "cj_idx1"]}
        rows = {nm: sb(f"r_{nm}", [128, 128], F32) for nm in GATHERS}
        w_bf = {nm: sb(f"b_{nm}", list(w_f[nm].shape), BF16) for nm in w_f}
        biT = sb("biT", [128, 128], BF16)
        ciT = sb("ciT", [128, 128], BF16)
        bjT = sb("bjT", [128, 256], BF16)
        cjT = sb("cjT", [128, 256], BF16)
        A_f32 = [sb(f"Af{g}", [128, 128], F32) for g in (0, 1)]
        B_bf = [sb(f"Bb{g}", [128, 256], BF16) for g in (0, 1)]
        Cb0 = [sb(f"C0_{g}", [128, 512], BF16) for g in (0, 1)]
        Cb1 = [sb(f"C1_{g}", [128, 514], BF16) for g in (0, 1)]
        attn_all = sb("attn_all", [128, NBLK * 256], F32)
        scores_all = sb("scores_all", [128, NBLK * 256], F32)
        tmp_b = [[sb(f"tmp{g}_{b_}", [128, MB * 256], BF16) for b_ in (0, 1)]
                 for g in (0, 1)]
        z1_b = [[sb(f"z1{g}_{b_}", [128, MB * 256], BF16) for b_ in (0, 1)]
                for g in (0, 1)]
        e2r = [sb(f"e2r{k}", [128, 512], BF16) for k in range(3)]
        tp = [ent(nc.psum_tensor(f"tp{k}", [128, 512], F32)).ap() for k in range(3)]
        scp = [ent(nc.psum_tensor(f"scp{k}", [128, 256], F32)).ap() for k in range(2)]

        s_sp = ent(nc.semaphore("s_sp"))
        s_aq = ent(nc.semaphore("s_aq"))
        s_aq2 = ent(nc.semaphore("s_aq2"))
        s_gq = ent(nc.semaphore("s_gq"))
        s_pe = ent(nc.semaphore("s_pe"))
        s_act = ent(nc.semaphore("s_act"))
        s_dve = ent(nc.semaphore("s_dve"))
        s_pool = ent(nc.semaphore("s_pool"))
        SEM = {"sp": s_sp, "aq": s_aq, "gq": s_gq, "pe": s_pe, "act": s_act,
               "dve": s_dve, "pool": s_pool}

        def tt1_emit(e, eng_name, blk, g, parity):
            """tmp[i*256+n] = C[255-(mb0+i)+n] + B[n] for one parity half.

            Even-i batch reads Cb1 (C data at +1) so the bf16 element base
            (256 - mb0 - i) stays 4-byte aligned; odd-i reads Cb0.
            """
            mb0 = blk * MB
            ctile, coff = ((Cb1[g], 256 - mb0) if parity == 0 else (Cb0[g], 254 - mb0))
            in0 = sub_ap(ctile, coff, [ctile.ap[0], [-2, MB // 2], [1, 256]])
            in1 = sub_ap(B_bf[g], 0, [B_bf[g].ap[0], [0, MB // 2], [1, 256]])
            tmp_ap = tmp_b[g][blk % 2]
            out = sub_ap(tmp_ap, parity * 256, [tmp_ap.ap[0], [512, MB // 2], [1, 256]])
            e.wait_ge(s_act, dr_tick[(g, "C1")])
            if blk >= 2:
                # engines pipeline: even same-engine consumers of the blk-2
                # pass-2 ops need explicit completion waits
                e.wait_ge(s_act, act_tsa(blk - 2, g, A_I[g][-1]))
                e.wait_ge(s_pool, pool_tsg(blk - 2, g, G_I[-1]))
                e.wait_ge(s_dve, dve_tsv(blk - 2, g, V_I[g][-1]))
            eng = nc.vector if eng_name == "dve" else nc.gpsimd
            eng.tensor_tensor(out, in0, in1, op=AL.add).then_inc(
                s_dve if eng_name == "dve" else s_pool, 1
            )

        def ts_emit(e, eng_name, blk, g, i):
            """z1 = relu(tmp + A[:, m]) for one m."""
            m_local = blk * MB + i
            src = tmp_b[g][blk % 2][:, i * 256 : (i + 1) * 256]
            dst = z1_b[g][blk % 2][:, i * 256 : (i + 1) * 256]
            if i % 2 == 0:
                e.wait_ge(s_dve, dve_tt1e(blk, g))
            else:
                e.wait_ge(s_pool, pool_tt1o(blk, g))
            if blk >= 2:
                e.wait_ge(s_pe, pe_mm2g1((blk - 2) * NCH + i // 2))
            bias = A_f32[g][:, m_local : m_local + 1]
            if eng_name == "a":
                nc.scalar.activation(dst, src, AF.Relu, bias=bias, scale=1.0).then_inc(s_act, 1)
            elif eng_name == "v":
                nc.vector.tensor_scalar(dst, src, bias, 0.0, op0=AL.add, op1=AL.max).then_inc(s_dve, 1)
            else:
                nc.gpsimd.tensor_scalar(dst, src, bias, 0.0, op0=AL.add, op1=AL.max).then_inc(s_pool, 1)

        with nc.Block() as block:

            @block.sync
            def _(sync):
                for nm, dram in [("w1bi", d_w1bi), ("w1ci", d_w1ci),
                                 ("w1bj", d_w1bj), ("w1cj", d_w1cj),
                                 ("w1p0", d_w1p[0]), ("w1p1", d_w1p[1]),
                                 ("w2_0", d_w2[0]), ("w2_1", d_w2[1]),
                                 ("w3m0", d_w3m[0]), ("w3m1", d_w3m[1]),
                                 ("w3m2", d_w3m[2]), ("w3m3", d_w3m[3])]:
                    sync.dma_start(out=w_f[nm], in_=dram).then_inc(s_sp, 16)
                sync.dma_start(out=ident, in_=d_ident).then_inc(s_sp, 16)
                for blk in range(NBLK):
                    sync.wait_ge(s_pool, pool_res(blk))
                    bs = slice(blk * 256, (blk + 1) * 256)
                    sync.dma_start(
                        out=mh_dram_ap(d_out, blk * MB), in_=scores_all[:, bs]
                    ).then_inc(s_sp, 16)
                sync.wait_ge(s_sp, 16 * (len(SP_LOADS) + NBLK))

            @block.scalar
            def _(scalar):
                for nm, dram in [("bi_idx", d_bi_idx), ("ci_idx", d_ci_idx),
                                 ("bj_idx0", d_bj_idx[0:128, :]),
                                 ("bj_idx1", d_bj_idx[128:256, :]),
                                 ("cj_idx0", d_cj_idx[0:128, :]),
                                 ("cj_idx1", d_cj_idx[128:256, :])]:
                    scalar.dma_start(out=idx_sb[nm], in_=dram).then_inc(s_aq, 16)
                scalar.dma_start(out=w_f["epos0"], in_=d_epos[0]).then_inc(s_aq2, 16)
                scalar.dma_start(out=w_f["epos1"], in_=d_epos[1]).then_inc(s_aq2, 16)
                scalar.dma_start(
                    out=attn_all,
                    in_=sub_ap(d_attn, 0, [[N, MB], [M * N, H], [MB * N, NBLK], [1, N]]),
                ).then_inc(s_aq2, 16)
                # HWDGE fans out over several queues, so only aggregate
                # semaphore values are deterministic: wait for ALL loads once
                scalar.wait_ge(s_sp, 16 * len(SP_LOADS))
                scalar.wait_ge(s_aq2, 48)
                for nm in CASTS:
                    nc.scalar.activation(w_bf[nm], w_f[nm], AF.Copy).then_inc(s_act, 1)
                xplan = [(biT[:, :], tp[0][:, 0:128]), (ciT[:, :], tp[0][:, 128:256]),
                         (bjT[:, 0:128], tp[0][:, 256:384]), (bjT[:, 128:256], tp[0][:, 384:512]),
                         (cjT[:, 0:128], tp[1][:, 0:128]), (cjT[:, 128:256], tp[1][:, 128:256])]
                for k, (dst, src) in enumerate(xplan):
                    scalar.wait_ge(s_pe, k + 1)
                    nc.scalar.activation(dst, src, AF.Copy).then_inc(s_act, 1)
                for g in (0, 1):
                    ab = tp[2] if g == 0 else tp[0]
                    cps = tp[1] if g == 0 else tp[2]
                    scalar.wait_ge(s_pe, 6 + (2 if g == 0 else 7))
                    nc.scalar.activation(A_f32[g], ab[:, 0:128], AF.Copy).then_inc(s_act, 1)
                    scalar.wait_ge(s_pe, 6 + (4 if g == 0 else 9))
                    nc.scalar.activation(B_bf[g], ab[:, 128:384], AF.Copy).then_inc(s_act, 1)
                    scalar.wait_ge(s_pe, 6 + (5 if g == 0 else 10))
                    nc.scalar.activation(Cb0[g], cps[:, :], AF.Copy).then_inc(s_act, 1)
                    nc.scalar.activation(Cb1[g][:, 1:513], cps[:, :], AF.Copy).then_inc(s_act, 1)
                for blk in range(NBLK):
                    for g in (0, 1):
                        for i in A_I[g]:
                            ts_emit(scalar, "a", blk, g, i)
                    for c in range(NCH):
                        cg = blk * NCH + c
                        scalar.wait_ge(s_pe, pe_mm2g1(cg))
                        nc.scalar.activation(e2r[cg % 3], tp[cg % 3][:, :], AF.Relu).then_inc(s_act, 1)
                    scalar.wait_ge(s_pe, pe_mm3b(blk * NCH + NCH - 1))
                    bs = slice(blk * 256, (blk + 1) * 256)
                    nc.scalar.activation(scores_all[:, bs], scp[blk % 2][:, :], AF.Copy).then_inc(s_act, 1)

            @block.gpsimd
            def _(gpsimd):
                gpsimd.wait_ge(s_aq, 96)  # all 6 idx loads
                for nm, table, idx in [("bi", d_ebi, "bi_idx"), ("ci", d_eci, "ci_idx"),
                                       ("bj0", d_ebj, "bj_idx0"), ("bj1", d_ebj, "bj_idx1"),
                                       ("cj0", d_ecj, "cj_idx0"), ("cj1", d_ecj, "cj_idx1")]:
                    nc.gpsimd.indirect_dma_start(
                        out=rows[nm],
                        out_offset=None,
                        in_=table,
                        in_offset=bass.IndirectOffsetOnAxis(ap=idx_sb[idx][:, :1], axis=0),
                    ).then_inc(s_gq, 16)
                for blk in range(NBLK):
                    if blk > 0:
                        bs = slice((blk - 1) * 256, blk * 256)
                        gpsimd.wait_ge(s_act, act_scored(blk - 1))
                        if blk == 1:
                            gpsimd.wait_ge(s_aq2, 48)
                        nc.gpsimd.tensor_tensor(
                            scores_all[:, bs], scores_all[:, bs], attn_all[:, bs], op=AL.add
                        ).then_inc(s_pool, 1)
                    for g in (0, 1):
                        tt1_emit(gpsimd, "pool", blk, g, 1)
                        for i in G_I:
                            ts_emit(gpsimd, "g", blk, g, i)
                bs = slice((NBLK - 1) * 256, NBLK * 256)
                gpsimd.wait_ge(s_act, act_scored(NBLK - 1))
                nc.gpsimd.tensor_tensor(
                    scores_all[:, bs], scores_all[:, bs], attn_all[:, bs], op=AL.add
                ).then_inc(s_pool, 1)

            @block.vector
            def _(vector):
                for blk in range(NBLK):
                    for g in (0, 1):
                        tt1_emit(vector, "dve", blk, g, 0)
                        for i in V_I[g]:
                            ts_emit(vector, "v", blk, g, i)

            @block.tensor
            def _(tensor):
                tensor.wait_ge(s_sp, 16 * len(SP_LOADS))
                xsrc = [("bi", tp[0], 0), ("ci", tp[0], 128), ("bj0", tp[0], 256),
                        ("bj1", tp[0], 384), ("cj0", tp[1], 0), ("cj1", tp[1], 128)]
                tensor.wait_ge(s_gq, 96)  # all gathers (completions unordered)
                for nm, ps, off in xsrc:
                    nc.tensor.matmul(ps[:, off : off + 128], lhsT=rows[nm], rhs=ident,
                                     start=True, stop=True, skip_group_check=True).then_inc(s_pe, 1)
                for g in (0, 1):
                    ab = tp[2] if g == 0 else tp[0]
                    cps = tp[1] if g == 0 else tp[2]
                    gs = slice(g * 64, (g + 1) * 64)
                    if g == 1:
                        tensor.wait_ge(s_act, xd_tick[3])  # tp[0] transpose reads done
                    tensor.wait_ge(s_act, max(cast_tick["w1ci"], xd_tick[1]))
                    nc.tensor.matmul(ab[:, 0:128], lhsT=w_bf["w1bi"][gs, :], rhs=biT[gs, :],
                                     start=True, stop=False, skip_group_check=True).then_inc(s_pe, 1)
                    nc.tensor.matmul(ab[:, 0:128], lhsT=w_bf["w1ci"][gs, :], rhs=ciT[gs, :],
                                     start=False, stop=True, skip_group_check=True).then_inc(s_pe, 1)
                    tensor.wait_ge(s_act, max(cast_tick["w1cj"], xd_tick[5]))
                    nc.tensor.matmul(ab[:, 128:384], lhsT=w_bf["w1bj"][gs, :], rhs=bjT[gs, :],
                                     start=True, stop=False, skip_group_check=True).then_inc(s_pe, 1)
                    nc.tensor.matmul(ab[:, 128:384], lhsT=w_bf["w1cj"][gs, :], rhs=cjT[gs, :],
                                     start=False, stop=True, skip_group_check=True).then_inc(s_pe, 1)
                    # g1's C psum (tp[2]) must also wait for the g0 A/B
                    # drains that read tp[2]
                    tensor.wait_ge(s_act, max(cast_tick[f"epos{g}"], xd_tick[5],
                                              dr_tick[(0, "B")] if g == 1 else 0))
                    nc.tensor.matmul(cps[:, :], lhsT=w_bf[f"w1p{g}"], rhs=w_bf[f"epos{g}"],
                                     start=True, stop=True, skip_group_check=True).then_inc(s_pe, 1)
                for blk in range(NBLK):
                    for c in range(NCH):
                        cg = blk * NCH + c
                        if cg == 0:
                            tensor.wait_ge(s_act, ACT_SETUP)
                        waits = {}
                        for g in (0, 1):
                            for i in (2 * c, 2 * c + 1):
                                sem, tk = ts_tick(blk, g, i)
                                waits[sem] = max(waits.get(sem, 0), tk)
                        for sem, tk in waits.items():
                            tensor.wait_ge(SEM[sem], tk)
                        if cg >= 3:
                            tensor.wait_ge(s_act, act_e2d(cg - 3))
                        cs = slice(c * 512, (c + 1) * 512)
                        e2ps = tp[cg % 3]
                        nc.tensor.matmul(e2ps[0:64, :], lhsT=w_bf["w2_0"],
                                         rhs=z1_b[0][blk % 2][:, cs],
                                         start=True, stop=True, skip_group_check=True).then_inc(s_pe, 1)
                        nc.tensor.matmul(e2ps[64:128, :], lhsT=w_bf["w2_1"],
                                         rhs=z1_b[1][blk % 2][:, cs],
                                         start=True, stop=True, tile_position=(0, 64),
                                         skip_group_check=True).then_inc(s_pe, 1)
                        tensor.wait_ge(s_act, act_e2d(cg))
                        for j in (0, 1):
                            m = 2 * c + j
                            grp = m // 4
                            nc.tensor.matmul(
                                scp[blk % 2][32 * grp : 32 * grp + 32, :],
                                lhsT=w_bf[f"w3m{m % 4}"],
                                rhs=e2r[cg % 3][:, j * 256 : (j + 1) * 256],
                                start=(m % 4 == 0), stop=(m % 4 == 3),
                                tile_position=(0, 32 * grp),
                                skip_group_check=True).then_inc(s_pe, 1)

        nc.all_engine_barrier()
    return nc


def _get_nc():
    global _BUILT
    if _BUILT is None:
        _BUILT = _build_nc()
    return _BUILT


def _prep_inputs(attention_scores, b_seq, c_seq, e_pos, e_bi, e_bj, e_ci, e_cj, w1_e, w2_e, w3_e):
    """Host-side sharding + pure relayout (no arithmetic on values)."""
    f32 = np.float32
    attention_scores = np.ascontiguousarray(attention_scores, dtype=f32)
    b_seq = np.ascontiguousarray(b_seq, dtype=np.int32)
    c_seq = np.ascontiguousarray(c_seq, dtype=np.int32)
    e_pos = np.asarray(e_pos, dtype=f32)
    w1_e = np.asarray(w1_e, dtype=f32)
    w2_e = np.asarray(w2_e, dtype=f32)
    w3_e = np.asarray(w3_e, dtype=f32)

    ebi_t = np.ascontiguousarray(np.asarray(e_bi, f32).reshape(N_MB + 1, H * BS))
    ebj_t = np.ascontiguousarray(np.asarray(e_bj, f32).reshape(N_MB + 1, H * BS))
    eci_t = np.ascontiguousarray(np.asarray(e_ci, f32).reshape(N_C + 2, H * CS))
    ecj_t = np.ascontiguousarray(np.asarray(e_cj, f32).reshape(N_C + 2, H * CS))

    w1bi = np.zeros((128, 128), f32)
    w1ci = np.zeros((128, 128), f32)
    w1bj = np.zeros((128, 128), f32)
    w1cj = np.zeros((128, 128), f32)
    w1p = np.zeros((2, 128, 128), f32)
    w2b = np.zeros((2, 128, 64), f32)
    for g in (0, 1):
        for hl in range(4):
            h = 4 * g + hl
            r0 = g * 64 + hl * 16
            w1bi[r0 : r0 + 16, hl * 32 : hl * 32 + 32] = w1_e[32:48, :, h]
            w1ci[r0 : r0 + 16, hl * 32 : hl * 32 + 32] = w1_e[64:80, :, h]
            w1bj[r0 : r0 + 16, hl * 32 : hl * 32 + 32] = w1_e[48:64, :, h]
            w1cj[r0 : r0 + 16, hl * 32 : hl * 32 + 32] = w1_e[80:96, :, h]
            w1p[g, hl * 32 : hl * 32 + 32, hl * 32 : hl * 32 + 32] = w1_e[0:32, :, h]
            w2b[g, hl * 32 : hl * 32 + 32, hl * 16 : hl * 16 + 16] = w2_e[:, :, h]

    # layer-3 variants: w3m[v][g*64+hl*16+l, v*8+h] = w3_e[l, 4g+hl]
    w3m = np.zeros((4, 128, 32), f32)
    for v in range(4):
        for g in (0, 1):
            for hl in range(4):
                h = 4 * g + hl
                w3m[v, g * 64 + hl * 16 : g * 64 + hl * 16 + 16, v * 8 + h] = w3_e[:, h]

    # per-m-half shifted+reversed position table:
    # epos_sh[mh][g, hl*32+p, t] = e_pos[511 - t + 128*mh, 4g+hl, p]  (0 if OOR)
    epos_sh = []
    for mh in (0, 1):
        m0 = 128 * mh
        arr = np.zeros((512, H, P), f32)
        t = np.arange(512)
        d = 511 - t + m0
        valid = (d >= 0) & (d < 512)
        arr[t[valid]] = e_pos[d[valid]]
        a = arr.reshape(512, 2, 4, P).transpose(1, 2, 3, 0).reshape(2, 128, 512)
        epos_sh.append(np.ascontiguousarray(a))

    in_maps = []
    for c in range(NCORES):
        b, mh = c // 2, c % 2
        m0 = 128 * mh
        in_maps.append(
            {
                "t_attn": np.ascontiguousarray(attention_scores[b, :, m0 : m0 + M, :]),
                "t_ebi": ebi_t,
                "t_ebj": ebj_t,
                "t_eci": eci_t,
                "t_ecj": ecj_t,
                "t_bi_idx": np.ascontiguousarray(b_seq[b, m0 : m0 + M].reshape(M, 1)),
                "t_ci_idx": np.ascontiguousarray(c_seq[b, m0 : m0 + M].reshape(M, 1)),
                "t_bj_idx": np.ascontiguousarray(b_seq[b].reshape(N, 1)),
                "t_cj_idx": np.ascontiguousarray(c_seq[b].reshape(N, 1)),
                "t_w1bi": w1bi,
                "t_ident": np.eye(128, dtype=f32),
                "t_w1ci": w1ci,
                "t_w1bj": w1bj,
                "t_w1cj": w1cj,
                "t_w1p": w1p,
                "t_w2": w2b,
                "t_w3m": w3m,
                "t_epos": epos_sh[mh],
            }
        )
    return in_maps


def kernel(**inputs):
    global LAST_RESULT
    from concourse.bass_utils import run_bass_kernel_spmd

    in_maps = _prep_inputs(**inputs)
    nc = _get_nc()
    res = run_bass_kernel_spmd(nc, in_maps, core_ids=list(range(NCORES)))
    LAST_RESULT = res
    out = np.empty((B, H, S, S), np.float32)
    for c in range(NCORES):
        b, mh = c // 2, c % 2
        m0 = 128 * mh
        out[b, :, m0 : m0 + M, :] = res.results[c]["t_out"]
    return out


if __name__ == "__main__":
    nc = _get_nc()
    print("built ok")
